# revision 1
# baseline (speedup 1.0000x reference)
"""Trainium2 Bass kernel for ExpertMLP: out = relu(x @ W_fc.T)^2 @ W_proj.T.

Sharding: 4-way tokens x 2-way hidden across 8 NeuronCores.
Each core computes a partial out[t_shard, :] contracted over its hidden
half; the host sums the two hidden halves while unsharding.

Per-core kernel (T_S=2048 tokens, HID_S=2048 hidden, DIM=1024):
  mm1: h^T[j, t] = W_fcT_shard.T-chunks @ xT-chunks   (PSUM accum over d)
  act: relu^2 (ScalarE relu PSUM->SBUF, VectorE square)
  mm2: out[t, d] = h^T-chunks.T @ W_projT_shard-chunks (PSUM accum over j)
All matmuls run as float32r (relaxed fp32, full PE rate at N>=256).
Both weight shards stay resident in SBUF; only x/out stream per t-chunk.
"""

import numpy as np

import concourse.mybir as mybir
import concourse.tile as tile
from concourse import bacc
from concourse import bass_utils

T, DIM, HID = 8192, 1024, 4096
N_CORES = 8
TOK_WAYS, HID_WAYS = 4, 2
T_S = T // TOK_WAYS        # 2048 tokens per core
HID_S = HID // HID_WAYS    # 2048 hidden units per core
P = 128
F32 = mybir.dt.float32
F32R = mybir.dt.float32r

T_CHUNK = 256              # mm1 moving free dim (tokens per inner chunk)
MM2_N = 512                # mm2 moving free dim (output-dim slice)

KD = DIM // P              # 8 contraction chunks for mm1
JC = HID_S // P            # 16 j-chunks (also mm2 contraction chunks)
TC = T_S // T_CHUNK        # 8 t-chunks
TSUB = T_CHUNK // P        # 2 psum token sub-chunks per t-chunk
DH = DIM // MM2_N          # 2 output-dim slices


def build_nc(reps: int = 1):
    nc = bacc.Bacc("TRN2", target_bir_lowering=False, debug=False)
    xT = nc.dram_tensor("xT", [DIM, T_S], F32, kind="ExternalInput")
    wfcT = nc.dram_tensor("wfcT", [DIM, HID_S], F32, kind="ExternalInput")
    wprojT = nc.dram_tensor("wprojT", [HID_S, DIM], F32, kind="ExternalInput")
    out = nc.dram_tensor("out", [T_S, DIM], F32, kind="ExternalOutput")

    xT_r = xT.ap().rearrange("(o p) t -> p o t", p=P).bitcast(F32R)
    wfcT_r = wfcT.ap().rearrange("(o p) h -> p o h", p=P).bitcast(F32R)
    wprojT_r = wprojT.ap().rearrange("(o p) d -> p o d", p=P).bitcast(F32R)
    out_r = out.ap().rearrange("(o p) d -> p o d", p=P)

    with tile.TileContext(nc) as tc:
        with (
            tc.tile_pool(name="weights", bufs=1) as wpool,
            tc.tile_pool(name="xin", bufs=2) as xpool,
            tc.tile_pool(name="hact", bufs=1) as hpool,
            tc.tile_pool(name="tmp", bufs=3) as tpool,
            tc.tile_pool(name="outp", bufs=3) as opool,
            tc.tile_pool(name="ps_h", bufs=2, space="PSUM") as ps_h_pool,
            tc.tile_pool(name="ps_o", bufs=2, space="PSUM") as ps_o_pool,
        ):
            wfc_sb = wpool.tile([P, KD, HID_S], F32R)
            wproj_sb = wpool.tile([P, JC, DIM], F32R)
            # Split resident-weight loads so early mm1 j-chunks aren't gated
            # on the full 16MB.
            H_SPLIT = 256
            for js in range(HID_S // H_SPLIT):
                sl = slice(js * H_SPLIT, (js + 1) * H_SPLIT)
                nc.sync.dma_start(wfc_sb[:, :, sl], wfcT_r[:, :, sl])
            for js in range(4):
                sl = slice(js * (JC // 4), (js + 1) * (JC // 4))
                nc.sync.dma_start(wproj_sb[:, sl, :], wprojT_r[:, sl, :])

            def body(_iv=None):
                for tc_i in range(TC):
                    tsl = slice(tc_i * T_CHUNK, (tc_i + 1) * T_CHUNK)
                    x_sb = xpool.tile([P, KD, T_CHUNK], F32R, tag="x")
                    nc.sync.dma_start(x_sb[:], xT_r[:, :, tsl])

                    h_sb = hpool.tile([P, JC, T_CHUNK], F32R, tag="h")
                    for j in range(JC):
                        ps = ps_h_pool.tile([P, T_CHUNK], F32, tag="psh")
                        for k in range(KD):
                            nc.tensor.matmul(
                                ps[:],
                                lhsT=wfc_sb[:, k, j * P:(j + 1) * P],
                                rhs=x_sb[:, k, :],
                                start=(k == 0),
                                stop=(k == KD - 1),
                            )
                        relu_t = tpool.tile([P, T_CHUNK], F32, tag="relu")
                        nc.scalar.activation(
                            relu_t[:], ps[:], mybir.ActivationFunctionType.Relu
                        )
                        nc.vector.tensor_mul(
                            out=h_sb[:, j, :], in0=relu_t[:], in1=relu_t[:]
                        )

                    for ts_i in range(TSUB):
                        for dh in range(DH):
                            ps_o = ps_o_pool.tile([P, MM2_N], F32, tag="pso")
                            for j in range(JC):
                                nc.tensor.matmul(
                                    ps_o[:],
                                    lhsT=h_sb[:, j, ts_i * P:(ts_i + 1) * P],
                                    rhs=wproj_sb[:, j, dh * MM2_N:(dh + 1) * MM2_N],
                                    start=(j == 0),
                                    stop=(j == JC - 1),
                                )
                            o_sb = opool.tile([P, MM2_N], F32, tag="o")
                            nc.vector.tensor_copy(out=o_sb[:], in_=ps_o[:])
                            nc.sync.dma_start(
                                out_r[:, tc_i * TSUB + ts_i,
                                      dh * MM2_N:(dh + 1) * MM2_N],
                                o_sb[:],
                            )

            if reps == 1:
                body()
            else:
                with tc.For_i(0, reps, 1) as iv:
                    body(iv)

    nc.compile()
    return nc


_NC_CACHE = {}


def _get_nc(reps: int = 1):
    if reps not in _NC_CACHE:
        _NC_CACHE[reps] = build_nc(reps)
    return _NC_CACHE[reps]


def make_in_maps(x, W_fc, W_proj):
    xT = np.ascontiguousarray(x.T)  # [DIM, T]
    in_maps = []
    for c in range(N_CORES):
        tok, hid = c // HID_WAYS, c % HID_WAYS
        hsl = slice(hid * HID_S, (hid + 1) * HID_S)
        in_maps.append({
            "xT": np.ascontiguousarray(xT[:, tok * T_S:(tok + 1) * T_S]),
            "wfcT": np.ascontiguousarray(W_fc[hsl, :].T),
            "wprojT": np.ascontiguousarray(W_proj[:, hsl].T),
        })
    return in_maps


def assemble_out(results):
    out = np.empty((T, DIM), dtype=np.float32)
    for tok in range(TOK_WAYS):
        acc = results[tok * HID_WAYS]["out"].copy()
        for hid in range(1, HID_WAYS):
            acc += results[tok * HID_WAYS + hid]["out"]
        out[tok * T_S:(tok + 1) * T_S] = acc
    return out


def kernel(x, W_fc, W_proj):
    assert x.shape == (T, DIM) and W_fc.shape == (HID, DIM) and W_proj.shape == (DIM, HID)
    nc = _get_nc(reps=1)
    in_maps = make_in_maps(
        np.asarray(x, np.float32),
        np.asarray(W_fc, np.float32),
        np.asarray(W_proj, np.float32),
    )
    res = bass_utils.run_bass_kernel_spmd(nc, in_maps, core_ids=list(range(N_CORES)))
    return assemble_out(res.results)


# revision 2
# speedup vs baseline: 4.7222x; 4.7222x over previous
"""Trainium2 Bass kernel for ExpertMLP: out = relu(x @ W_fc.T)^2 @ W_proj.T.

Sharding: 4-way tokens x 2-way hidden across 8 NeuronCores.
Each core computes a partial out[t_shard, :] contracted over its hidden
half; the host sums the two hidden halves while unsharding.

Per-core kernel (T_S=2048 tokens, HID_S=2048 hidden, DIM=1024):
  mm1: h^T[j, t] = W_fcT_shard-chunks.T @ xT-chunks   (PSUM accum over d)
  act: relu^2 (ScalarE relu PSUM->SBUF fp16, VectorE square)
  mm2: out[t, d] = h^T-chunks.T @ W_projT_shard-chunks (PSUM accum over j)

Matmul operands are fp16 (full PE rate, fast weight load, fp32 PSUM
accumulation; end-to-end scale-relative error ~5e-4). Host casts inputs to
fp16; output stays fp32. Both weight shards stay resident in SBUF; only
x/out stream per 512-token chunk.
"""

import numpy as np

import concourse.mybir as mybir
import concourse.tile as tile
from concourse import bacc
from concourse import bass_utils

T, DIM, HID = 8192, 1024, 4096
N_CORES = 8
TOK_WAYS, HID_WAYS = 4, 2
T_S = T // TOK_WAYS        # 2048 tokens per core
HID_S = HID // HID_WAYS    # 2048 hidden units per core
P = 128
F32 = mybir.dt.float32
F16 = mybir.dt.float16

T_CHUNK = 512              # mm1 moving free dim (tokens per inner chunk)
MM2_N = 512                # mm2 moving free dim (output-dim slice)

KD = DIM // P              # 8 contraction chunks for mm1
JC = HID_S // P            # 16 j-chunks (also mm2 contraction chunks)
TC = T_S // T_CHUNK        # 4 t-chunks
TSUB = T_CHUNK // P        # 4 psum token sub-chunks per t-chunk
DH = DIM // MM2_N          # 2 output-dim slices


def build_nc(reps: int = 1):
    nc = bacc.Bacc("TRN2", target_bir_lowering=False, debug=False)
    xT = nc.dram_tensor("xT", [DIM, T_S], F16, kind="ExternalInput")
    wfcT = nc.dram_tensor("wfcT", [DIM, HID_S], F16, kind="ExternalInput")
    wprojT = nc.dram_tensor("wprojT", [HID_S, DIM], F16, kind="ExternalInput")
    out = nc.dram_tensor("out", [T_S, DIM], F32, kind="ExternalOutput")

    xT_r = xT.ap().rearrange("(o p) t -> p o t", p=P)
    wfcT_r = wfcT.ap().rearrange("(o p) h -> p o h", p=P)
    wprojT_r = wprojT.ap().rearrange("(o p) d -> p o d", p=P)
    out_r = out.ap().rearrange("(o p) d -> p o d", p=P)

    with tile.TileContext(nc) as tc:
        with (
            tc.tile_pool(name="weights", bufs=1) as wpool,
            tc.tile_pool(name="xin", bufs=2) as xpool,
            tc.tile_pool(name="hact", bufs=2) as hpool,
            tc.tile_pool(name="tmp", bufs=3) as tpool,
            tc.tile_pool(name="outp", bufs=3) as opool,
            tc.tile_pool(name="ps_h", bufs=3, space="PSUM") as ps_h_pool,
            tc.tile_pool(name="ps_o", bufs=3, space="PSUM") as ps_o_pool,
        ):
            wfc_sb = wpool.tile([P, KD, HID_S], F16)
            wproj_sb = wpool.tile([P, JC, DIM], F16)
            # Split resident-weight loads so early mm1 j-chunks aren't gated
            # on the full 8MB.
            H_SPLIT = 256
            for js in range(HID_S // H_SPLIT):
                sl = slice(js * H_SPLIT, (js + 1) * H_SPLIT)
                nc.sync.dma_start(wfc_sb[:, :, sl], wfcT_r[:, :, sl])
            for js in range(4):
                sl = slice(js * (JC // 4), (js + 1) * (JC // 4))
                nc.sync.dma_start(wproj_sb[:, sl, :], wprojT_r[:, sl, :])

            def body(_iv=None):
                for tc_i in range(TC):
                    tsl = slice(tc_i * T_CHUNK, (tc_i + 1) * T_CHUNK)
                    x_sb = xpool.tile([P, KD, T_CHUNK], F16, tag="x")
                    nc.sync.dma_start(x_sb[:], xT_r[:, :, tsl])

                    h_sb = hpool.tile([P, JC, T_CHUNK], F16, tag="h")
                    for j in range(JC):
                        ps = ps_h_pool.tile([P, T_CHUNK], F32, tag="psh")
                        for k in range(KD):
                            nc.tensor.matmul(
                                ps[:],
                                lhsT=wfc_sb[:, k, j * P:(j + 1) * P],
                                rhs=x_sb[:, k, :],
                                start=(k == 0),
                                stop=(k == KD - 1),
                            )
                        relu_t = tpool.tile([P, T_CHUNK], F16, tag="relu")
                        nc.scalar.activation(
                            relu_t[:], ps[:], mybir.ActivationFunctionType.Relu
                        )
                        nc.vector.tensor_mul(
                            out=h_sb[:, j, :], in0=relu_t[:], in1=relu_t[:]
                        )

                    for ts_i in range(TSUB):
                        for dh in range(DH):
                            ps_o = ps_o_pool.tile([P, MM2_N], F32, tag="pso")
                            for j in range(JC):
                                nc.tensor.matmul(
                                    ps_o[:],
                                    lhsT=h_sb[:, j, ts_i * P:(ts_i + 1) * P],
                                    rhs=wproj_sb[:, j, dh * MM2_N:(dh + 1) * MM2_N],
                                    start=(j == 0),
                                    stop=(j == JC - 1),
                                )
                            o_sb = opool.tile([P, MM2_N], F32, tag="o")
                            nc.vector.tensor_copy(out=o_sb[:], in_=ps_o[:])
                            nc.sync.dma_start(
                                out_r[:, tc_i * TSUB + ts_i,
                                      dh * MM2_N:(dh + 1) * MM2_N],
                                o_sb[:],
                            )

            if reps == 1:
                body()
            else:
                with tc.For_i(0, reps, 1) as iv:
                    body(iv)

    nc.compile()
    return nc


_NC_CACHE = {}


def _get_nc(reps: int = 1):
    if reps not in _NC_CACHE:
        _NC_CACHE[reps] = build_nc(reps)
    return _NC_CACHE[reps]


def make_in_maps(x, W_fc, W_proj):
    xT = np.ascontiguousarray(x.T.astype(np.float16))  # [DIM, T]
    wfcT16 = {}
    wprojT16 = {}
    for hid in range(HID_WAYS):
        hsl = slice(hid * HID_S, (hid + 1) * HID_S)
        wfcT16[hid] = np.ascontiguousarray(W_fc[hsl, :].T.astype(np.float16))
        wprojT16[hid] = np.ascontiguousarray(W_proj[:, hsl].T.astype(np.float16))
    in_maps = []
    for c in range(N_CORES):
        tok, hid = c // HID_WAYS, c % HID_WAYS
        in_maps.append({
            "xT": np.ascontiguousarray(xT[:, tok * T_S:(tok + 1) * T_S]),
            "wfcT": wfcT16[hid],
            "wprojT": wprojT16[hid],
        })
    return in_maps


def assemble_out(results):
    out = np.empty((T, DIM), dtype=np.float32)
    for tok in range(TOK_WAYS):
        acc = results[tok * HID_WAYS]["out"].copy()
        for hid in range(1, HID_WAYS):
            acc += results[tok * HID_WAYS + hid]["out"]
        out[tok * T_S:(tok + 1) * T_S] = acc
    return out


def kernel(x, W_fc, W_proj):
    assert x.shape == (T, DIM) and W_fc.shape == (HID, DIM) and W_proj.shape == (DIM, HID)
    nc = _get_nc(reps=1)
    in_maps = make_in_maps(
        np.asarray(x, np.float32),
        np.asarray(W_fc, np.float32),
        np.asarray(W_proj, np.float32),
    )
    res = bass_utils.run_bass_kernel_spmd(nc, in_maps, core_ids=list(range(N_CORES)))
    return assemble_out(res.results)
